# revision 6
# baseline (speedup 1.0000x reference)
"""Trainium2 Bass kernel for nn_Attention (B=2, N=2048, DIM=2048, H=16, HD=128).

Sharding: 8 cores = 2 batches x 4 head-groups (4 heads each). Each core:
  - QKV projection (token-partition layout); lead-in streams the first
    token-tile's x as a dedicated [128,16,128] load racing the 16 weight
    tiles across both hw queues (coef/wout on a third, gpsimd-triggered
    queue), so the PE starts ~11us in and tile 0 finishes at the HBM
    bandwidth floor
  - per-head RMSNorm (2 wide ACT squares + segmented DVE reduce; gammas
    folded into host-precomputed RoPE coefficients), RoPE on host-
    deinterleaved q/k channels (packed fp16, 2x DVE), PE-transpose of Q/K
    to [hd, n] (tile 15's transpose deferred into the first attention slot;
    tile 15's qkv psum is drained to SBUF by one DVE copy so phase 2's
    score psum banks free up ~4us earlier)
  - S^T = K^T.T @ Q^T scores (m on partitions), exp on ACT in PAIRS
    ([128,2,512] psum -> one activation over 1024 elems, amortizing the
    fixed access/dispatch cost; ACT drops below the PE's slot pace), no max
    subtraction needed (scores are O(few) for this regime), softmax sums
    via a binary DVE add-tree over exp pairs feeding ONE 128-wide-ones
    matmul per slot, PV accumulation (lag 3 behind scores), normalization
    via reciprocal_approx_fast, output projection interleaved into the
    following chunk's head loops, fp16 partials to HBM
  - the last slot processes m-tiles rotated [14,15,0..13] so its tail
    never waits on the final exps
  - host sums the 4 head-group partials per batch.
"""

import sys

import numpy as np

sys.path.insert(0, "/opt/trn_rl_repo")

import ml_dtypes  # noqa: E402

import concourse.bass as bass  # noqa: E402
import concourse.tile as tile  # noqa: E402
from concourse import bacc  # noqa: E402
from concourse import mybir  # noqa: E402
from concourse.masks import make_identity  # noqa: E402

B, N, DIM, H, HD = 2, 2048, 2048, 16, 128
NCORES = 8
GROUPS = NCORES // B  # 4 head-groups
HPC = H // GROUPS  # 4 heads per core
CPC = HPC * HD  # 512 channels per core
EPS = 1e-5
SCALE = 1.0 / float(np.sqrt(HD))
EXP_OFF = -7.0  # keeps exp(s) in fp16 range (max observed score*scale ~16); cancels in softmax

NT = N // 128  # 16 token tiles
DT = DIM // 128  # 16 contraction tiles
NJ = N // 512  # 4 n-chunks
XTILES = 4  # leading token tiles with dedicated per-tile x loads

F32 = mybir.dt.float32
BF16 = mybir.dt.float16  # fp16: 8x finer mantissa than bf16, same PE rate
NPBF16 = np.float16
AF = mybir.ActivationFunctionType


def _emit(tc: "tile.TileContext"):
    nc = tc.nc
    xT = nc.dram_tensor("xT", [DIM, N], BF16, kind="ExternalInput")
    wqkvT = nc.dram_tensor("wqkvT", [DIM, 3 * CPC], BF16, kind="ExternalInput")
    woutT = nc.dram_tensor("woutT", [CPC, DIM], BF16, kind="ExternalInput")
    coef = nc.dram_tensor("coef", [N, 8, 2 * HD], BF16, kind="ExternalInput")
    outp = nc.dram_tensor("outp", [N, DIM], BF16, kind="ExternalOutput")

    with (
        tc.tile_pool(name="const", bufs=1) as const,
        tc.tile_pool(name="persist", bufs=1) as persist,
        # qr outlives phase 1: tile 15's transpose is deferred into phase 2
        tc.tile_pool(name="qr", bufs=3) as qrp,
    ):
        ident = const.tile([128, 128], BF16)
        make_identity(nc, ident)
        ones128 = const.tile([128, 128], BF16)
        nc.vector.memset(ones128, 1.0)
        eps_sb = const.tile([128, 1], F32)
        nc.vector.memset(eps_sb, EPS)
        expoff_sb = const.tile([128, 1], F32)
        nc.vector.memset(expoff_sb, EXP_OFF)
        # warm the ACT exp table while the lead-in is DMA-bound, so the
        # phase-2 transition doesn't pay the 1.3us table load
        warm = const.tile([1, 1], F32)
        nc.scalar.activation(out=warm, in_=eps_sb[0:1, :], func=AF.Exp)

        # persistent activations, split per 512-token chunk so phase 2 can
        # start before the whole of phase 1 has drained
        QT = [persist.tile([128, HPC, 512], BF16, tag=f"QT{j}", name=f"QT{j}") for j in range(NJ)]
        KT = [persist.tile([128, HPC, 512], BF16, tag=f"KT{j}", name=f"KT{j}") for j in range(NJ)]
        V = [persist.tile([128, 4, CPC], BF16, tag=f"V{j}", name=f"V{j}") for j in range(NJ)]
        O = [persist.tile([128, HPC, 512], BF16, tag=f"O{j}", name=f"O{j}") for j in range(NJ)]
        wout_sb = const.tile([128, HPC, DIM], BF16)
        qr_tiles = {}

        # ---------------- phase 1: QKV + rmsnorm + rope + transpose ------
        with (
            tc.tile_pool(name="wq", bufs=1) as wqp,
            tc.tile_pool(name="xs", bufs=1) as xsp,
            tc.tile_pool(name="cf", bufs=3) as cfp,
            tc.tile_pool(name="qn", bufs=1) as qnp,
            tc.tile_pool(name="scr", bufs=2) as scrp,
            tc.tile_pool(name="qkv_ps", bufs=2, space="PSUM") as qkvps,
            tc.tile_pool(name="tr_ps", bufs=2, space="PSUM") as trps,
        ):
            wq_sb = [None] * DT
            xts = [None] * XTILES
            xgen = {}
            cfs = {}
            xTr = xT.rearrange("(dd p) n -> p dd n", p=128)

            def load_wq(t, eng):
                wt = wqp.tile([128, 3 * CPC], BF16, tag=f"wq{t}", name="wt")
                eng.dma_start(out=wt, in_=wqkvT[t * 128 : (t + 1) * 128, :])
                wq_sb[t] = wt

            def load_xt(i, eng):
                # one leading token tile: all 16 contraction rows for 128
                # tokens, so tile 0 compute races the weight stream
                xb = xsp.tile([128, DT, 128], BF16, tag=f"xt{i}", name="xb")
                eng.dma_start(out=xb, in_=xTr[:, :, i * 128 : (i + 1) * 128])
                xts[i] = xb

            def load_xgen(g):
                # tiles 4g..4g+3 as two dd-halves on the sync queue
                for half in range(2):
                    xb = xsp.tile([128, 8, 4 * 128], BF16, tag=f"xg{g % 2}h{half}", name="xb")
                    nc.sync.dma_start(
                        out=xb,
                        in_=xTr[:, half * 8 : half * 8 + 8, g * 512 : (g + 1) * 512],
                    )
                    xgen.setdefault(g, [None, None])[half] = xb

            def load_cf(i):
                cf = cfp.tile([128, 8, 2 * HD], BF16, tag="cf", name="cf")
                nc.gpsimd.dma_start(out=cf, in_=coef[i * 128 : (i + 1) * 128, :, :])
                cfs[i] = cf

            # lead-in: first token tile's x + weight tiles race across both
            # hw queues in consumption order; coef/wout stream on the
            # gpsimd-triggered queue
            load_wq(0, nc.scalar)
            load_xt(0, nc.sync)
            load_xt(1, nc.gpsimd)
            for t in range(1, DT):
                load_wq(t, nc.sync if t % 2 == 1 else nc.scalar)
            load_xt(2, nc.gpsimd)
            load_xt(3, nc.gpsimd)
            load_cf(0)
            load_cf(1)

            def transposes(i):
                qr = qr_tiles[i % 3]
                for qk in range(2):
                    trp = trps.tile([128, CPC], BF16, name="trp")
                    for h in range(HPC):
                        hsl = slice(h * HD, (h + 1) * HD)
                        nc.tensor.transpose(
                            trp[:, hsl], qr[:, qk, hsl], ident
                        )
                    tgt = (QT if qk == 0 else KT)[i // 4]
                    dst = tgt[:, :, (i % 4) * 128 : (i % 4 + 1) * 128]
                    nc.vector.tensor_copy(
                        out=dst,
                        in_=trp.rearrange("p (h n) -> p h n", h=HPC),
                    )

            for i in range(NT):
                g = i // 4
                if i == 0:
                    load_xgen(1)
                elif i == 2:
                    nc.gpsimd.dma_start(
                        out=wout_sb, in_=woutT.rearrange("(h p) d -> p h d", p=128)
                    )
                elif i == 4:
                    load_xgen(2)
                elif i == 8:
                    load_xgen(3)
                if i + 2 < NT:
                    load_cf(i + 2)
                ps = qkvps.tile([128, 3, CPC], F32, name="ps")
                for d in range(DT):
                    if i < XTILES:
                        lhsT = xts[i][:, d, :]
                    else:
                        lhsT = xgen[g][d // 8][:, d % 8, (i % 4) * 128 : (i % 4 + 1) * 128]
                    for c in range(3):
                        nc.tensor.matmul(
                            ps[:, c, :],
                            lhsT=lhsT,
                            rhs=wq_sb[d][:, c * CPC : (c + 1) * CPC],
                            start=(d == 0),
                            stop=(d == DT - 1),
                        )

                # transposes lag 2 tiles so the PE never waits on the
                # rmsnorm/rope chain of the tile being transposed
                if i > 1:
                    transposes(i - 2)

                # V straight to SBUF (bf16)
                nc.vector.tensor_copy(out=V[i // 4][:, i % 4, :], in_=ps[:, 2, :])

                # tile 15: drain the q/k psum to SBUF with one wide copy so
                # phase 2's score psum banks only wait ~2us, not the whole
                # rmsnorm read chain
                if i == NT - 1:
                    qk15 = scrp.tile([128, 2, CPC], F32, tag="qk15", bufs=1)
                    nc.vector.tensor_copy(out=qk15, in_=ps[:, 0:2, :])
                    src_qk = qk15
                else:
                    src_qk = ps

                # rmsnorm: 2 wide squares on ACT + one segmented DVE reduce
                # (replaces 8 accum-squares: far less ACT serial time)
                ssq = scrp.tile([128, 8], F32, tag="ssq")
                sq = scrp.tile([128, 2, CPC], BF16, tag="sq", bufs=1)
                for qk in range(2):
                    nc.scalar.activation(
                        out=sq[:, qk, :], in_=src_qk[:, qk, :], func=AF.Square
                    )
                nc.vector.tensor_reduce(
                    out=ssq,
                    in_=sq.rearrange("p a (s hd) -> p (a s) hd", hd=HD),
                    axis=mybir.AxisListType.X,
                    op=mybir.AluOpType.add,
                )
                rstd = scrp.tile([128, 8], F32, tag="rstd")
                nc.scalar.activation(
                    rstd, ssq, AF.Sqrt, bias=eps_sb, scale=1.0 / HD
                )
                nc.vector.reciprocal(rstd, rstd)

                qn = qnp.tile([128, 2, CPC], BF16, name="qn")
                for qk in range(2):
                    for h in range(HPC):
                        hsl = slice(h * HD, (h + 1) * HD)
                        nc.vector.tensor_scalar_mul(
                            out=qn[:, qk, hsl],
                            in0=src_qk[:, qk, hsl],
                            scalar1=rstd[:, qk * HPC + h : qk * HPC + h + 1],
                        )

                # rope; q/k channels are host-permuted to [evens|odds] per
                # head, so every operand here is packed fp16 (2x DVE rate)
                cf = cfs.pop(i)
                qr = qrp.tile([128, 2, CPC], BF16, name="qr")
                qr_tiles[i % 3] = qr
                for qk in range(2):
                    base = qk * 4
                    dq = qn[:, qk, :].rearrange("p (h z c) -> p h z c", z=2, c=HD // 2)
                    x0 = dq[:, :, 0, :]
                    x1 = dq[:, :, 1, :]
                    rot = qr[:, qk, :].rearrange("p (h z c) -> p h z c", z=2, c=HD // 2)

                    def cf3(k):
                        return cf[:, base + k, :].rearrange("p (h c) -> p h c", c=HD // 2)

                    ta = scrp.tile([128, HPC, HD // 2], BF16, tag="ta", bufs=1)
                    tb = scrp.tile([128, HPC, HD // 2], BF16, tag="tb", bufs=1)
                    nc.vector.tensor_mul(ta, x0, cf3(0))
                    nc.vector.tensor_mul(tb, x1, cf3(1))
                    nc.vector.tensor_sub(rot[:, :, 0, :], ta, tb)
                    tc2 = scrp.tile([128, HPC, HD // 2], BF16, tag="tc2", bufs=1)
                    td = scrp.tile([128, HPC, HD // 2], BF16, tag="td", bufs=1)
                    nc.vector.tensor_mul(tc2, x0, cf3(2))
                    nc.vector.tensor_mul(td, x1, cf3(3))
                    nc.vector.tensor_add(rot[:, :, 1, :], tc2, td)

            transposes(NT - 2)
            # tile 15's transpose is deferred into phase 2 slot (0,0): its
            # rmsnorm/rope chain would otherwise stall the PE ~9us here

        # ------------- phase 2+3: attention + output projection ----------
        with (
            tc.tile_pool(name="ps2", bufs=1, space="PSUM") as sps,
            tc.tile_pool(name="op_ps", bufs=2, space="PSUM") as opsp,
            tc.tile_pool(name="o_ps", bufs=2, space="PSUM") as ops_,
            tc.tile_pool(name="es", bufs=1) as esp,
            tc.tile_pool(name="lv", bufs=1) as lvp,
            tc.tile_pool(name="invsb", bufs=2) as invsbp,
            tc.tile_pool(name="ob", bufs=4) as obp,
        ):
            # outproj emitted as fine-grained items interleaved into the
            # m-loops of the following chunk's head slots, so the PE's
            # per-step slack absorbs it instead of serializing
            op_state = {}

            def emit_op_item(item):
                if item[0] == "mm":
                    _, jj, it, dch, hh = item
                    if hh == 0:
                        op_state["ps"] = opsp.tile(
                            [128, 512], F32, tag="op", name="op_ps"
                        )
                    nc.tensor.matmul(
                        op_state["ps"],
                        lhsT=O[jj][:, hh, it * 128 : (it + 1) * 128],
                        rhs=wout_sb[:, hh, dch * 512 : (dch + 1) * 512],
                        start=(hh == 0),
                        stop=(hh == HPC - 1),
                    )
                else:
                    _, jj, it, dch = item
                    nsl = slice((4 * jj + it) * 128, (4 * jj + it + 1) * 128)
                    ob = obp.tile([128, 512], BF16, tag="ob", name="ob")
                    nc.vector.tensor_copy(out=ob, in_=op_state["ps"])
                    (nc.sync if dch % 2 == 0 else nc.scalar).dma_start(
                        out=outp[nsl, dch * 512 : (dch + 1) * 512], in_=ob
                    )

            def push_op_items(jj):
                for it in range(4):
                    for dch in range(4):
                        for hh in range(HPC):
                            opq.append(("mm", jj, it, dch, hh))
                        opq.append(("cp", jj, it, dch))

            OPS_AT_POS = {5: 2, 6: 2, 7: 2, 8: 2, 9: 2, 10: 2, 11: 2, 12: 2,
                          13: 2, 14: 1, 15: 1}

            def emit_tr15():
                i = NT - 1
                qr = qr_tiles[i % 3]
                for qk in range(2):
                    trp = opsp.tile([128, CPC], BF16, tag="op", name="trp")
                    for hh in range(HPC):
                        hsl = slice(hh * HD, (hh + 1) * HD)
                        nc.tensor.transpose(trp[:, hsl], qr[:, qk, hsl], ident)
                    tgt = (QT if qk == 0 else KT)[i // 4]
                    nc.vector.tensor_copy(
                        out=tgt[:, :, (i % 4) * 128 : (i % 4 + 1) * 128],
                        in_=trp.rearrange("p (h n) -> p h n", h=HPC),
                    )

            opq = []
            pend_a = None
            pend_b = None
            for j in range(NJ):
                for h in range(HPC):
                    last_slot = j == NJ - 1 and h == HPC - 1
                    # the last slot computes m 14,15 first so its tail (which
                    # has no following slot to hide in) never waits on ACT
                    ms = [14, 15] + list(range(14)) if last_slot else list(range(NT))
                    o_ps = ops_.tile([128, 512], F32, tag="o", name="o_ps")
                    espair = {}
                    l1s = {}
                    l2s = {}

                    def pv(pos, o_ps=o_ps, espair=espair, ms=ms, h=h):
                        m = ms[pos]
                        nc.tensor.matmul(
                            o_ps,
                            lhsT=V[m // 4][:, m % 4, h * HD : (h + 1) * HD],
                            rhs=espair[pos // 2][:, pos % 2, :],
                            start=(pos == 0),
                            stop=(pos == NT - 1),
                        )

                    def tail_a(pv=pv, l2s=l2s):
                        # PE part of the slot tail: last three PV matmuls
                        # (their exps are 3+ steps old by now)
                        pv(NT - 3)
                        pv(NT - 2)
                        pv(NT - 1)

                    def tail_b(j=j, h=h, o_ps=o_ps, l2s=l2s):
                        # softmax denominator: final tree level + one
                        # 128-wide-ones matmul, then normalize
                        l3 = lvp.tile([128, 2, 512], BF16, tag="l3")
                        nc.vector.tensor_add(l3, l2s[0], l2s[1])
                        sin_ = lvp.tile([128, 512], BF16, tag="sin")
                        nc.vector.tensor_add(sin_, l3[:, 0, :], l3[:, 1, :])
                        sum_ps = opsp.tile([128, 512], F32, tag="op", name="sum_ps")
                        nc.tensor.matmul(
                            sum_ps, lhsT=ones128, rhs=sin_, start=True, stop=True
                        )
                        inv = invsbp.tile([128, 512], F32, tag="invsb", name="inv")
                        nc.vector.reciprocal_approx_fast(out=inv, in_=sum_ps)
                        nc.vector.tensor_mul(O[j][:, h, :], o_ps, inv)
                        if h == HPC - 1:
                            push_op_items(j)

                    for pos in range(NT):
                        m = ms[pos]
                        p = pos // 2
                        if pos % 2 == 0:
                            ps_pair = sps.tile(
                                [128, 2, 512], F32, tag=f"s{p % 2}", name="s_ps"
                            )
                        nc.tensor.matmul(
                            ps_pair[:, pos % 2, :],
                            lhsT=KT[m // 4][:, h, (m % 4) * 128 : (m % 4 + 1) * 128],
                            rhs=QT[j][:, h, :],
                            start=True,
                            stop=True,
                        )
                        if pos % 2 == 1:
                            # one exp instruction per psum PAIR: amortizes the
                            # ACT access+dispatch overhead, ACT drops below
                            # the PE slot pace
                            es = esp.tile(
                                [128, 2, 512], BF16, tag=f"es{p % 4}", name="es"
                            )
                            nc.scalar.activation(
                                es, ps_pair, AF.Exp, scale=SCALE, bias=expoff_sb
                            )
                            espair[p] = es
                            if p % 2 == 1:
                                l1 = lvp.tile(
                                    [128, 2, 512], BF16, tag=f"l1{(p // 2) % 2}"
                                )
                                nc.vector.tensor_add(l1, espair[p - 1], espair[p])
                                l1s[p // 2] = l1
                                if p == 3 or p == 7:
                                    l2 = lvp.tile(
                                        [128, 2, 512], BF16, tag=f"l2{p // 4}"
                                    )
                                    nc.vector.tensor_add(
                                        l2, l1s[p // 2 - 1], l1s[p // 2]
                                    )
                                    l2s[p // 4] = l2
                        if pos == 1 and pend_a is not None:
                            pend_a()
                            pend_a = None
                        if pos == 4 and pend_b is not None:
                            pend_b()
                            pend_b = None
                        if pos >= 3:
                            pv(pos - 3)
                            for _ in range(OPS_AT_POS.get(pos, 0)):
                                if opq:
                                    emit_op_item(opq.pop(0))
                            if j == 0 and h == 0 and pos == 13:
                                emit_tr15()
                    pend_a = tail_a
                    pend_b = tail_b

            pend_a()
            pend_b()
            while opq:
                emit_op_item(opq.pop(0))


_NC = None


def _get_nc():
    global _NC
    if _NC is None:
        nc = bacc.Bacc()
        with tile.TileContext(nc) as tc:
            _emit(tc)
        if not nc.is_finalized():
            nc.finalize()
        _NC = nc
    return _NC


def _deint(W):
    # reorder each head's output channels to [evens | odds] so rope pairs
    # (2i, 2i+1) become (i, i+64): packed DVE access on-device; scores are
    # invariant to any per-head channel permutation applied to both q and k
    W = W.reshape(HPC, HD, DIM)
    W = np.concatenate([W[:, 0::2, :], W[:, 1::2, :]], axis=1)
    return W.reshape(HPC * HD, DIM)


def _prep_core(x, Wqkv, q_gamma, k_gamma, Wout, cos, sin, b, hg):
    hsl = slice(hg * CPC, (hg + 1) * CPC)
    Wq = _deint(Wqkv[0 * H * HD : 1 * H * HD][hsl])
    Wk = _deint(Wqkv[1 * H * HD : 2 * H * HD][hsl])
    Wv = Wqkv[2 * H * HD : 3 * H * HD][hsl]
    wqkvT = np.ascontiguousarray(np.concatenate([Wq, Wk, Wv], 0).T)
    woutT = np.ascontiguousarray(Wout[:, hsl].T)

    def c4(a):  # [N, 64] -> [N, 256] tiled over the 4 heads
        return np.tile(a, (1, HPC))

    qe, qo = q_gamma[0::2], q_gamma[1::2]
    ke, ko = k_gamma[0::2], k_gamma[1::2]
    cb, sb = cos[b], sin[b]  # [N, 64]
    coef = np.stack(
        [
            c4(cb * qe), c4(sb * qo), c4(sb * qe), c4(cb * qo),
            c4(cb * ke), c4(sb * ko), c4(sb * ke), c4(cb * ko),
        ],
        axis=1,
    ).astype(np.float16)  # [N, 8, 256]
    return {
        "xT": np.ascontiguousarray(x[b].T).astype(NPBF16),
        "wqkvT": wqkvT.astype(NPBF16),
        "woutT": woutT.astype(NPBF16),
        "coef": np.ascontiguousarray(coef),
    }


def prep_in_maps(x, Wqkv, q_gamma, k_gamma, Wout, freqs):
    x = np.asarray(x, np.float32)
    Wqkv = np.asarray(Wqkv, np.float32)
    Wout = np.asarray(Wout, np.float32)
    q_gamma = np.asarray(q_gamma, np.float32)
    k_gamma = np.asarray(k_gamma, np.float32)
    freqs = np.asarray(freqs, np.float32)
    cos = freqs[..., 0]
    sin = freqs[..., 1]
    return [
        _prep_core(x, Wqkv, q_gamma, k_gamma, Wout, cos, sin, c // GROUPS, c % GROUPS)
        for c in range(NCORES)
    ]


def gather(parts):
    out = np.empty((B, N, DIM), np.float32)
    for b in range(B):
        acc = parts[b * GROUPS].astype(np.float32)
        for g in range(1, GROUPS):
            acc = acc + parts[b * GROUPS + g]
        out[b] = acc
    return out


def kernel(x, Wqkv, q_gamma, k_gamma, Wout, freqs):
    from concourse.bass_utils import run_bass_kernel_spmd

    nc = _get_nc()
    in_maps = prep_in_maps(x, Wqkv, q_gamma, k_gamma, Wout, freqs)
    res = run_bass_kernel_spmd(nc, in_maps, list(range(NCORES)))
    parts = [res.results[c]["outp"] for c in range(NCORES)]
    return gather(parts)


# revision 12
# speedup vs baseline: 1.0104x; 1.0104x over previous
"""Trainium2 Bass kernel for nn_Attention (B=2, N=2048, DIM=2048, H=16, HD=128).

Sharding: 8 cores = 2 batches x 4 head-groups (4 heads each). Each core:
  - QKV projection (token-partition layout) over tiles in order
    [15, 0, 1, .., 14] so the last-processed tile's rmsnorm/rope chain
    (which drains ~9us past its matmuls) belongs to tile 14, whose
    K-columns phase 2 touches last; the lead-in streams tile 15's x as a
    dedicated [128,16,128] load racing the 16 weight tiles across both hw
    queues (coef on the gpsimd-triggered queue, buffer-rotation throttled
    so it cannot steal lead-in bandwidth; wout deferred)
  - per-head RMSNorm (2 wide ACT squares + segmented DVE reduce; gammas
    folded into host-precomputed RoPE coefficients), RoPE on host-
    deinterleaved q/k channels (packed fp16, 2x DVE), PE-transpose of Q/K
    to [hd, n] (tile 14's transpose deferred into the first attention
    slot; the last tile's qkv psum is drained to SBUF by one DVE copy so
    phase 2's score psum banks free up ~4us earlier)
  - S^T = K^T.T @ Q^T scores (m on partitions), exp on ACT (no max needed:
    scores are O(few) for this regime), softmax sums via a binary DVE
    add-tree over exp tiles feeding ONE 128-wide-ones matmul per slot,
    PV accumulation, normalization via reciprocal_approx_fast, output
    projection interleaved into the following chunk's ACT-bound head
    loops, fp16 partials to HBM
  - the last slot processes m-tiles rotated [14,15,0..13] so its tail
    never waits on the final exps
  - host sums the 4 head-group partials per batch.
"""

import sys

import numpy as np

sys.path.insert(0, "/opt/trn_rl_repo")

import ml_dtypes  # noqa: E402

import concourse.bass as bass  # noqa: E402
import concourse.tile as tile  # noqa: E402
from concourse import bacc  # noqa: E402
from concourse import mybir  # noqa: E402
from concourse.masks import make_identity  # noqa: E402

B, N, DIM, H, HD = 2, 2048, 2048, 16, 128
NCORES = 8
GROUPS = NCORES // B  # 4 head-groups
HPC = H // GROUPS  # 4 heads per core
CPC = HPC * HD  # 512 channels per core
EPS = 1e-5
SCALE = 1.0 / float(np.sqrt(HD))
EXP_OFF = -7.0  # keeps exp(s) in fp16 range (max observed score*scale ~16); cancels in softmax

NT = N // 128  # 16 token tiles
DT = DIM // 128  # 16 contraction tiles
NJ = N // 512  # 4 n-chunks

F32 = mybir.dt.float32
BF16 = mybir.dt.float16  # fp16: 8x finer mantissa than bf16, same PE rate
NPBF16 = np.float16
AF = mybir.ActivationFunctionType

# phase-1 token-tile processing order: tile 15 first so the last tile's
# post-matmul chain belongs to 14
P1_ORDER = [15] + list(range(15))


def _emit(tc: "tile.TileContext"):
    nc = tc.nc
    xT = nc.dram_tensor("xT", [DIM, N], BF16, kind="ExternalInput")
    wqkvT = nc.dram_tensor("wqkvT", [DIM, 3 * CPC], BF16, kind="ExternalInput")
    woutT = nc.dram_tensor("woutT", [CPC, DIM], BF16, kind="ExternalInput")
    coef = nc.dram_tensor("coef", [N, 8, 2 * HD], BF16, kind="ExternalInput")
    outp = nc.dram_tensor("outp", [N, DIM], BF16, kind="ExternalOutput")

    with (
        tc.tile_pool(name="const", bufs=1) as const,
        tc.tile_pool(name="persist", bufs=1) as persist,
        # qr outlives phase 1: tile 14's transpose is deferred into phase 2
        tc.tile_pool(name="qr", bufs=3) as qrp,
    ):
        ident = const.tile([128, 128], BF16)
        make_identity(nc, ident)
        ones128 = const.tile([128, 128], BF16)
        nc.vector.memset(ones128, 1.0)
        eps_sb = const.tile([128, 1], F32)
        nc.vector.memset(eps_sb, EPS)
        expoff_sb = const.tile([128, 1], F32)
        nc.vector.memset(expoff_sb, EXP_OFF)
        # warm the ACT exp table while the lead-in is DMA-bound, so the
        # phase-2 transition doesn't pay the 1.3us table load
        warm = const.tile([1, 1], F32)
        nc.scalar.activation(out=warm, in_=eps_sb[0:1, :], func=AF.Exp)

        # persistent activations, split per 512-token chunk so phase 2 can
        # start before the whole of phase 1 has drained
        QT = [persist.tile([128, HPC, 512], BF16, tag=f"QT{j}", name=f"QT{j}") for j in range(NJ)]
        KT = [persist.tile([128, HPC, 512], BF16, tag=f"KT{j}", name=f"KT{j}") for j in range(NJ)]
        V = [persist.tile([128, 4, CPC], BF16, tag=f"V{j}", name=f"V{j}") for j in range(NJ)]
        O = [persist.tile([128, HPC, 512], BF16, tag=f"O{j}", name=f"O{j}") for j in range(NJ)]
        wout_sb = const.tile([128, HPC, DIM], BF16)
        qr_tiles = {}

        # ---------------- phase 1: QKV + rmsnorm + rope + transpose ------
        with (
            tc.tile_pool(name="wq", bufs=1) as wqp,
            tc.tile_pool(name="xs", bufs=1) as xsp,
            tc.tile_pool(name="cf", bufs=3) as cfp,
            tc.tile_pool(name="qn", bufs=1) as qnp,
            tc.tile_pool(name="scr", bufs=2) as scrp,
            tc.tile_pool(name="qkv_ps", bufs=2, space="PSUM") as qkvps,
            tc.tile_pool(name="tr_ps", bufs=2, space="PSUM") as trps,
        ):
            wq_sb = [None] * DT
            xts = {}
            xgen = {}
            cfs = {}
            xTr = xT.rearrange("(dd p) n -> p dd n", p=128)

            def load_wq(t, eng):
                wt = wqp.tile([128, 3 * CPC], BF16, tag=f"wq{t}", name="wt")
                eng.dma_start(out=wt, in_=wqkvT[t * 128 : (t + 1) * 128, :])
                wq_sb[t] = wt

            def load_xt(i, eng):
                # one leading token tile: all 16 contraction rows for 128
                # tokens, so the first tile's compute races the weight stream
                xb = xsp.tile([128, DT, 128], BF16, tag=f"xt{i}", name="xb")
                eng.dma_start(out=xb, in_=xTr[:, :, i * 128 : (i + 1) * 128])
                xts[i] = xb

            xgen_off = {}

            def load_xgen(g, start_tile, n_tiles):
                # tiles of a 4-tile generation as two dd-halves on the sync
                # queue (FIFO keeps them behind the critical stream)
                off = start_tile * 128
                ntok = n_tiles * 128
                xgen_off[g] = off
                for half in range(2):
                    xb = xsp.tile([128, 8, ntok], BF16, tag=f"xg{g % 2}h{half}", name="xb")
                    nc.sync.dma_start(
                        out=xb,
                        in_=xTr[:, half * 8 : half * 8 + 8, off : off + ntok],
                    )
                    xgen.setdefault(g, [None, None])[half] = xb

            def load_cf(i):
                # gpsimd-triggered queue; the 3-buffer rotation throttles this
                # stream so it can never run ahead and steal lead-in bandwidth
                cf = cfp.tile([128, 8, 2 * HD], BF16, tag="cf", name="cf")
                nc.gpsimd.dma_start(out=cf, in_=coef[i * 128 : (i + 1) * 128, :, :])
                cfs[i] = cf

            # lead-in: first processed tile (15) races the weight tiles on
            # the two hw queues; everything else is either FIFO-queued behind
            # the critical stream or rotation-throttled on the gpsimd queue
            load_wq(0, nc.scalar)
            load_xt(15, nc.sync)
            for t in range(1, DT):
                load_wq(t, nc.sync if t % 2 == 1 else nc.scalar)
            load_cf(15)
            load_cf(0)
            load_xt(0, nc.scalar)
            load_xt(1, nc.sync)
            load_xgen(0, 2, 2)  # tiles 2,3

            def transposes(i):
                qr = qr_tiles.pop(i)
                for qk in range(2):
                    trp = trps.tile([128, CPC], BF16, name="trp")
                    for h in range(HPC):
                        hsl = slice(h * HD, (h + 1) * HD)
                        nc.tensor.transpose(
                            trp[:, hsl], qr[:, qk, hsl], ident
                        )
                    tgt = (QT if qk == 0 else KT)[i // 4]
                    dst = tgt[:, :, (i % 4) * 128 : (i % 4 + 1) * 128]
                    nc.vector.tensor_copy(
                        out=dst,
                        in_=trp.rearrange("p (h n) -> p h n", h=HPC),
                    )

            for k in range(NT):
                i = P1_ORDER[k]
                if k == 1:
                    load_xgen(1, 4, 4)
                elif k == 4:
                    load_xgen(2, 8, 4)
                elif k == 8:
                    load_xgen(3, 12, 3)  # tiles 12,13,14; 15 is the lead tile
                    nc.gpsimd.dma_start(
                        out=wout_sb, in_=woutT.rearrange("(h p) d -> p h d", p=128)
                    )
                if k + 2 < NT:
                    load_cf(P1_ORDER[k + 2])
                ps = qkvps.tile([128, 3, CPC], F32, name="ps")
                for d in range(DT):
                    if i in xts:
                        lhsT = xts[i][:, d, :]
                    else:
                        g = i // 4
                        col = i * 128 - xgen_off[g]
                        lhsT = xgen[g][d // 8][:, d % 8, col : col + 128]
                    for c in range(3):
                        nc.tensor.matmul(
                            ps[:, c, :],
                            lhsT=lhsT,
                            rhs=wq_sb[d][:, c * CPC : (c + 1) * CPC],
                            start=(d == 0),
                            stop=(d == DT - 1),
                        )

                # transposes lag 2 tiles so the PE never waits on the
                # rmsnorm/rope chain of the tile being transposed
                if k > 1:
                    transposes(P1_ORDER[k - 2])

                # V straight to SBUF (bf16)
                nc.vector.tensor_copy(out=V[i // 4][:, i % 4, :], in_=ps[:, 2, :])

                # last tile: drain the q/k psum to SBUF with one wide copy so
                # phase 2's score psum banks only wait ~2us, not the whole
                # rmsnorm read chain
                if k == NT - 1:
                    qkl = scrp.tile([128, 2, CPC], F32, tag="qkl", bufs=1)
                    nc.vector.tensor_copy(out=qkl, in_=ps[:, 0:2, :])
                    src_qk = qkl
                else:
                    src_qk = ps

                # rmsnorm: 2 wide squares on ACT + one segmented DVE reduce
                # (replaces 8 accum-squares: far less ACT serial time)
                ssq = scrp.tile([128, 8], F32, tag="ssq")
                sq = scrp.tile([128, 2, CPC], BF16, tag="sq", bufs=1)
                for qk in range(2):
                    nc.scalar.activation(
                        out=sq[:, qk, :], in_=src_qk[:, qk, :], func=AF.Square
                    )
                nc.vector.tensor_reduce(
                    out=ssq,
                    in_=sq.rearrange("p a (s hd) -> p (a s) hd", hd=HD),
                    axis=mybir.AxisListType.X,
                    op=mybir.AluOpType.add,
                )
                rstd = scrp.tile([128, 8], F32, tag="rstd")
                nc.scalar.activation(
                    rstd, ssq, AF.Sqrt, bias=eps_sb, scale=1.0 / HD
                )
                nc.vector.reciprocal(rstd, rstd)

                qn = qnp.tile([128, 2, CPC], BF16, name="qn")
                for qk in range(2):
                    for h in range(HPC):
                        hsl = slice(h * HD, (h + 1) * HD)
                        nc.vector.tensor_scalar_mul(
                            out=qn[:, qk, hsl],
                            in0=src_qk[:, qk, hsl],
                            scalar1=rstd[:, qk * HPC + h : qk * HPC + h + 1],
                        )

                # rope; q/k channels are host-permuted to [evens|odds] per
                # head, so every operand here is packed fp16 (2x DVE rate)
                cf = cfs.pop(i)
                qr = qrp.tile([128, 2, CPC], BF16, name="qr")
                qr_tiles[i] = qr
                for qk in range(2):
                    base = qk * 4
                    dq = qn[:, qk, :].rearrange("p (h z c) -> p h z c", z=2, c=HD // 2)
                    x0 = dq[:, :, 0, :]
                    x1 = dq[:, :, 1, :]
                    rot = qr[:, qk, :].rearrange("p (h z c) -> p h z c", z=2, c=HD // 2)

                    def cf3(kk):
                        return cf[:, base + kk, :].rearrange("p (h c) -> p h c", c=HD // 2)

                    ta = scrp.tile([128, HPC, HD // 2], BF16, tag="ta", bufs=1)
                    tb = scrp.tile([128, HPC, HD // 2], BF16, tag="tb", bufs=1)
                    nc.vector.tensor_mul(ta, x0, cf3(0))
                    nc.vector.tensor_mul(tb, x1, cf3(1))
                    nc.vector.tensor_sub(rot[:, :, 0, :], ta, tb)
                    tc2 = scrp.tile([128, HPC, HD // 2], BF16, tag="tc2", bufs=1)
                    td = scrp.tile([128, HPC, HD // 2], BF16, tag="td", bufs=1)
                    nc.vector.tensor_mul(tc2, x0, cf3(2))
                    nc.vector.tensor_mul(td, x1, cf3(3))
                    nc.vector.tensor_add(rot[:, :, 1, :], tc2, td)

            transposes(P1_ORDER[NT - 2])
            # tile 14's transpose is deferred into phase 2 slot (0,0): its
            # rmsnorm/rope chain would otherwise stall the PE ~9us here

        # ------------- phase 2+3: attention + output projection ----------
        with (
            tc.tile_pool(name="ps2", bufs=3, space="PSUM") as sps,
            tc.tile_pool(name="op_ps", bufs=2, space="PSUM") as opsp,
            tc.tile_pool(name="o_ps", bufs=2, space="PSUM") as ops_,
            tc.tile_pool(name="es", bufs=2) as esp,
            tc.tile_pool(name="lv", bufs=2) as lvp,
            tc.tile_pool(name="invsb", bufs=2) as invsbp,
            tc.tile_pool(name="ob", bufs=4) as obp,
        ):
            # outproj emitted as fine-grained items interleaved into the
            # (ACT-bound) m-loops of the following chunk's head slots, so
            # the PE's per-step slack absorbs it instead of serializing
            op_state = {}

            def emit_op_item(item):
                if item[0] == "mm":
                    _, jj, it, dch, hh = item
                    if hh == 0:
                        op_state["ps"] = opsp.tile(
                            [128, 512], F32, tag="op", name="op_ps"
                        )
                    nc.tensor.matmul(
                        op_state["ps"],
                        lhsT=O[jj][:, hh, it * 128 : (it + 1) * 128],
                        rhs=wout_sb[:, hh, dch * 512 : (dch + 1) * 512],
                        start=(hh == 0),
                        stop=(hh == HPC - 1),
                    )
                else:
                    _, jj, it, dch = item
                    nsl = slice((4 * jj + it) * 128, (4 * jj + it + 1) * 128)
                    ob = obp.tile([128, 512], BF16, tag="ob", name="ob")
                    nc.vector.tensor_copy(out=ob, in_=op_state["ps"])
                    (nc.sync if dch % 2 == 0 else nc.scalar).dma_start(
                        out=outp[nsl, dch * 512 : (dch + 1) * 512], in_=ob
                    )

            def push_op_items(jj):
                for it in range(4):
                    for dch in range(4):
                        for hh in range(HPC):
                            opq.append(("mm", jj, it, dch, hh))
                        opq.append(("cp", jj, it, dch))

            OPS_AT_POS = {4: 1, 5: 2, 6: 2, 7: 2, 8: 2, 9: 2, 10: 2, 11: 2,
                          12: 2, 13: 1, 14: 1, 15: 1}

            def emit_tr_deferred():
                i = P1_ORDER[NT - 1]
                qr = qr_tiles.pop(i)
                for qk in range(2):
                    trp = opsp.tile([128, CPC], BF16, tag="op", name="trp")
                    for hh in range(HPC):
                        hsl = slice(hh * HD, (hh + 1) * HD)
                        nc.tensor.transpose(trp[:, hsl], qr[:, qk, hsl], ident)
                    tgt = (QT if qk == 0 else KT)[i // 4]
                    nc.vector.tensor_copy(
                        out=tgt[:, :, (i % 4) * 128 : (i % 4 + 1) * 128],
                        in_=trp.rearrange("p (h n) -> p h n", h=HPC),
                    )

            opq = []
            pend_a = None
            pend_b = None
            for j in range(NJ):
                for h in range(HPC):
                    last_slot = j == NJ - 1 and h == HPC - 1
                    # the last slot computes m 14,15 first so its tail (which
                    # has no following slot to hide in) never waits on ACT
                    ms = [14, 15] + list(range(14)) if last_slot else list(range(NT))
                    o_ps = ops_.tile([128, 512], F32, tag="o", name="o_ps")
                    ess = {}
                    pas = {}
                    qds = {}
                    ocs = {}
                    hxs = {}

                    def pv(pos, o_ps=o_ps, ess=ess, ms=ms, h=h):
                        m = ms[pos]
                        nc.tensor.matmul(
                            o_ps,
                            lhsT=V[m // 4][:, m % 4, h * HD : (h + 1) * HD],
                            rhs=ess[pos],
                            start=(pos == 0),
                            stop=(pos == NT - 1),
                        )

                    def tail_a(pv=pv):
                        # PE part of the slot tail: last two PV matmuls
                        pv(NT - 2)
                        pv(NT - 1)

                    def tail_b(j=j, h=h, o_ps=o_ps, hxs=hxs):
                        # softmax denominator: ONE 128-wide-ones matmul over
                        # the fully tree-reduced exp tiles, then normalize
                        sum_ps = opsp.tile([128, 512], F32, tag="op", name="sum_ps")
                        nc.tensor.matmul(
                            sum_ps, lhsT=ones128, rhs=hxs[0], start=True, stop=True
                        )
                        inv = invsbp.tile([128, 512], F32, tag="invsb", name="inv")
                        nc.vector.reciprocal_approx_fast(out=inv, in_=sum_ps)
                        nc.vector.tensor_mul(O[j][:, h, :], o_ps, inv)
                        if h == HPC - 1:
                            push_op_items(j)

                    for pos in range(NT):
                        m = ms[pos]
                        s_ps = sps.tile([128, 512], F32, tag="s", name="s_ps")
                        nc.tensor.matmul(
                            s_ps,
                            lhsT=KT[m // 4][:, h, (m % 4) * 128 : (m % 4 + 1) * 128],
                            rhs=QT[j][:, h, :],
                            start=True,
                            stop=True,
                        )
                        es = esp.tile([128, 512], BF16, tag=f"es{pos % 4}", name="es")
                        nc.scalar.activation(
                            es, s_ps, AF.Exp, scale=SCALE, bias=expoff_sb
                        )
                        ess[pos] = es
                        # binary DVE reduce tree over exp tiles: one sums
                        # matmul per slot instead of two
                        if pos % 2 == 1:
                            pa = lvp.tile([128, 512], BF16, tag=f"pa{(pos // 2) % 2}", name="pa")
                            nc.vector.tensor_add(pa, ess[pos - 1], ess[pos])
                            pas[pos // 2] = pa
                        if pos % 4 == 3:
                            q = pos // 4
                            qds[q] = lvp.tile([128, 512], BF16, tag=f"qd{q % 2}", name="qd")
                            nc.vector.tensor_add(qds[q], pas[pos // 2 - 1], pas[pos // 2])
                            if q % 2 == 1:
                                ocs[q // 2] = lvp.tile(
                                    [128, 512], BF16, tag=f"oc{q // 2}", name="oc"
                                )
                                nc.vector.tensor_add(ocs[q // 2], qds[q - 1], qds[q])
                                if q == 3:
                                    hxs[0] = lvp.tile([128, 512], BF16, tag="hx", name="hx")
                                    nc.vector.tensor_add(hxs[0], ocs[0], ocs[1])
                        if pos == 1 and pend_a is not None:
                            pend_a()
                            pend_a = None
                        if pos == 4 and pend_b is not None:
                            pend_b()
                            pend_b = None
                        if pos >= 2:
                            pv(pos - 2)
                            for _ in range(OPS_AT_POS.get(pos, 0)):
                                if opq:
                                    emit_op_item(opq.pop(0))
                            if j == 0 and h == 0 and pos == 12:
                                emit_tr_deferred()
                    pend_a = tail_a
                    pend_b = tail_b

            pend_a()
            pend_b()
            while opq:
                emit_op_item(opq.pop(0))


_NC = None


def _get_nc():
    global _NC
    if _NC is None:
        nc = bacc.Bacc()
        with tile.TileContext(nc) as tc:
            _emit(tc)
        if not nc.is_finalized():
            nc.finalize()
        _NC = nc
    return _NC


def _deint(W):
    # reorder each head's output channels to [evens | odds] so rope pairs
    # (2i, 2i+1) become (i, i+64): packed DVE access on-device; scores are
    # invariant to any per-head channel permutation applied to both q and k
    W = W.reshape(HPC, HD, DIM)
    W = np.concatenate([W[:, 0::2, :], W[:, 1::2, :]], axis=1)
    return W.reshape(HPC * HD, DIM)


def _prep_core(x, Wqkv, q_gamma, k_gamma, Wout, cos, sin, b, hg):
    hsl = slice(hg * CPC, (hg + 1) * CPC)
    Wq = _deint(Wqkv[0 * H * HD : 1 * H * HD][hsl])
    Wk = _deint(Wqkv[1 * H * HD : 2 * H * HD][hsl])
    Wv = Wqkv[2 * H * HD : 3 * H * HD][hsl]
    wqkvT = np.ascontiguousarray(np.concatenate([Wq, Wk, Wv], 0).T)
    woutT = np.ascontiguousarray(Wout[:, hsl].T)

    def c4(a):  # [N, 64] -> [N, 256] tiled over the 4 heads
        return np.tile(a, (1, HPC))

    qe, qo = q_gamma[0::2], q_gamma[1::2]
    ke, ko = k_gamma[0::2], k_gamma[1::2]
    cb, sb = cos[b], sin[b]  # [N, 64]
    coef = np.stack(
        [
            c4(cb * qe), c4(sb * qo), c4(sb * qe), c4(cb * qo),
            c4(cb * ke), c4(sb * ko), c4(sb * ke), c4(cb * ko),
        ],
        axis=1,
    ).astype(np.float16)  # [N, 8, 256]
    return {
        "xT": np.ascontiguousarray(x[b].T).astype(NPBF16),
        "wqkvT": wqkvT.astype(NPBF16),
        "woutT": woutT.astype(NPBF16),
        "coef": np.ascontiguousarray(coef),
    }


def prep_in_maps(x, Wqkv, q_gamma, k_gamma, Wout, freqs):
    x = np.asarray(x, np.float32)
    Wqkv = np.asarray(Wqkv, np.float32)
    Wout = np.asarray(Wout, np.float32)
    q_gamma = np.asarray(q_gamma, np.float32)
    k_gamma = np.asarray(k_gamma, np.float32)
    freqs = np.asarray(freqs, np.float32)
    cos = freqs[..., 0]
    sin = freqs[..., 1]
    return [
        _prep_core(x, Wqkv, q_gamma, k_gamma, Wout, cos, sin, c // GROUPS, c % GROUPS)
        for c in range(NCORES)
    ]


def gather(parts):
    out = np.empty((B, N, DIM), np.float32)
    for b in range(B):
        acc = parts[b * GROUPS].astype(np.float32)
        for g in range(1, GROUPS):
            acc = acc + parts[b * GROUPS + g]
        out[b] = acc
    return out


def kernel(x, Wqkv, q_gamma, k_gamma, Wout, freqs):
    from concourse.bass_utils import run_bass_kernel_spmd

    nc = _get_nc()
    in_maps = prep_in_maps(x, Wqkv, q_gamma, k_gamma, Wout, freqs)
    res = run_bass_kernel_spmd(nc, in_maps, list(range(NCORES)))
    parts = [res.results[c]["outp"] for c in range(NCORES)]
    return gather(parts)
